# revision 9
# baseline (speedup 1.0000x reference)
"""Trainium2 Bass kernel for nn_ReadinMatrix (moe_routing).

Math (per sample b):
    readin_b = unique_readin[session[b]]            # [IN, RDIM]
    out[b]   = (state_in[b] @ readin_b) @ project   # [T*A, OUT]

Sharding: data-parallel over batch B across 8 cores (16 samples/core).
The per-sample weight is fused on device: W_b = readin_b @ project
([IN, OUT], 2 small matmuls), then out[b] = state[b] @ W_b runs as 16
accumulating matmuls per sample with contiguous DMA in both directions.

Host staging: the state shard is laid out feature-major ([IN, T*A] per
sample, the lhsT convention) so the contraction axis lands on SBUF
partitions without any on-device transpose; the readin gather+transpose
also happens host-side while building the per-core input maps.
"""

import numpy as np

import concourse.bass as bass
import concourse.mybir as mybir
import concourse.tile as tile
from concourse import bacc
from concourse.bass import ts
from concourse.bass_utils import run_bass_kernel_spmd

B = 128
T = 512
A = 2
TA = T * A          # 1024 tokens per sample
IN = 192
RDIM = 64
OUT = 256
N_CORES = 8
BPC = B // N_CORES  # 16 samples per core
MT = TA // 128      # 8 token tiles per sample

_nc_cache = {}
LAST_RESULTS = None  # BassKernelResults of the most recent run (for profiling)


def _build_nc(repeat=1):
    """Build the per-core Bass module. `repeat` re-runs the whole workload
    that many times inside one NEFF (used only for benchmarking: device
    exec time = (T_R - T_1) / (R - 1), cancelling dispatch overhead)."""
    if repeat in _nc_cache:
        return _nc_cache[repeat]

    f32 = mybir.dt.float32
    nc = bacc.Bacc(
        "TRN2", target_bir_lowering=False, debug=False, enable_asserts=False
    )
    stateT = nc.dram_tensor("stateT", [BPC, IN, TA], f32, kind="ExternalInput").ap()
    readinT = nc.dram_tensor("readinT", [BPC, RDIM, IN], f32, kind="ExternalInput").ap()
    proj = nc.dram_tensor("proj", [RDIM, OUT], f32, kind="ExternalInput").ap()
    out = nc.dram_tensor("out", [BPC, TA, OUT], f32, kind="ExternalOutput").ap()

    with tile.TileContext(nc) as tc, \
         tc.tile_pool(name="const", bufs=1) as cpool, \
         tc.tile_pool(name="w", bufs=3) as wpool, \
         tc.tile_pool(name="s", bufs=3) as spool, \
         tc.tile_pool(name="o", bufs=3) as opool, \
         tc.tile_pool(name="psw", bufs=2, space="PSUM") as pswpool, \
         tc.tile_pool(name="pso", bufs=4, space="PSUM") as psopool:

        proj_sb = cpool.tile([RDIM, OUT], f32)
        nc.sync.dma_start(proj_sb[:], proj)
        # all 16 samples' transposed readin matrices: [r, b, i], one 768KB DMA
        rT_sb = cpool.tile([RDIM, BPC, IN], f32)
        nc.sync.dma_start(rT_sb[:], readinT.rearrange("b r i -> r b i"))

        PAIR = 2  # samples per DMA batch (bigger transfers, fewer descriptors)
        for b0 in [p for _ in range(repeat) for p in range(0, BPC, PAIR)]:
            # ---- load state.T for PAIR samples ([IN, PAIR, TA]) ----
            s0 = spool.tile([128, PAIR, TA], f32, tag="s0")
            s1 = spool.tile([IN - 128, PAIR, TA], f32, tag="s1")
            nc.sync.dma_start(
                s0[:], stateT[b0:b0 + PAIR, 0:128, :].rearrange("b i t -> i b t"))
            nc.sync.dma_start(
                s1[:], stateT[b0:b0 + PAIR, 128:IN, :].rearrange("b i t -> i b t"))

            o_sb = opool.tile([128, PAIR, MT, OUT], f32, tag="o")
            for j in range(PAIR):
                b = b0 + j
                # ---- fuse W_b = readin_b @ project  ([IN, OUT], K=RDIM) ----
                ps_w0 = pswpool.tile([128, OUT], f32, tag="psw0")
                ps_w1 = pswpool.tile([IN - 128, OUT], f32, tag="psw1")
                nc.tensor.matmul(ps_w0[:], rT_sb[:, b, 0:128], proj_sb[:],
                                 start=True, stop=True)
                nc.tensor.matmul(ps_w1[:], rT_sb[:, b, 128:IN], proj_sb[:],
                                 start=True, stop=True)
                w0 = wpool.tile([128, OUT], f32, tag="w0")
                w1 = wpool.tile([IN - 128, OUT], f32, tag="w1")
                nc.scalar.copy(out=w0[:], in_=ps_w0[:])
                nc.scalar.copy(out=w1[:], in_=ps_w1[:])

                # ---- out_b = state_b @ W_b : 8 token tiles, K = 128 + 64 ----
                for mt in range(MT):
                    ps_o = psopool.tile([128, OUT], f32, tag="pso")
                    nc.tensor.matmul(ps_o[:], s0[:, j, ts(mt, 128)], w0[:],
                                     start=True, stop=False)
                    nc.tensor.matmul(ps_o[:], s1[:, j, ts(mt, 128)], w1[:],
                                     start=False, stop=True)
                    nc.vector.tensor_copy(out=o_sb[:, j, mt, :], in_=ps_o[:])
            # store on the ACT HWDGE ring so it doesn't queue behind loads
            nc.scalar.dma_start(
                out[b0:b0 + PAIR].rearrange("b (mt p) o -> p b mt o", p=128),
                o_sb[:])

    nc.compile()
    _nc_cache[repeat] = nc
    return nc


def _make_in_maps(state_in, session, unique_readin, project):
    state2d = np.ascontiguousarray(np.asarray(state_in), dtype=np.float32)
    state2d = state2d.reshape(B, TA, IN)
    session_np = np.asarray(session).astype(np.int64)
    table = np.ascontiguousarray(np.asarray(unique_readin), dtype=np.float32)
    proj_np = np.ascontiguousarray(np.asarray(project), dtype=np.float32)

    in_maps = []
    for c in range(N_CORES):
        sl = slice(c * BPC, (c + 1) * BPC)
        stT = np.ascontiguousarray(state2d[sl].transpose(0, 2, 1))
        rT = np.ascontiguousarray(table[session_np[sl]].transpose(0, 2, 1))
        in_maps.append({"stateT": stT, "readinT": rT, "proj": proj_np})
    return in_maps


def kernel(state_in, session, unique_readin, project):
    global LAST_RESULTS
    nc = _build_nc()
    in_maps = _make_in_maps(state_in, session, unique_readin, project)
    res = run_bass_kernel_spmd(nc, in_maps, core_ids=list(range(N_CORES)))
    LAST_RESULTS = res
    outs = [res.results[c]["out"].reshape(BPC, T, A, OUT) for c in range(N_CORES)]
    return np.concatenate(outs, axis=0)


# revision 16
# speedup vs baseline: 7.6752x; 7.6752x over previous
"""Trainium2 Bass kernel for nn_ReadinMatrix (moe_routing).

Math (per sample b):
    readin_b = unique_readin[session[b]]            # [IN, RDIM]
    out[b]   = (state_in[b] @ readin_b) @ project   # [T*A, OUT]

Sharding: data-parallel over batch B across 8 cores (16 samples/core).
The per-sample weight is fused on device: W_b = readin_b @ project
([IN, OUT], 2 small matmuls), then out[b] = state[b] @ W_b runs as 16
accumulating matmuls per sample with contiguous DMA in both directions.

Host staging: the state shard is laid out feature-major ([IN, T*A] per
sample, the lhsT convention) so the contraction axis lands on SBUF
partitions without any on-device transpose; the readin gather+transpose
also happens host-side while building the per-core input maps.
"""

import numpy as np

import concourse.bass as bass
import concourse.mybir as mybir
import concourse.tile as tile
from concourse import bacc
from concourse.bass import ts
from concourse.bass_utils import run_bass_kernel_spmd

B = 128
T = 512
A = 2
TA = T * A          # 1024 tokens per sample
IN = 192
RDIM = 64
OUT = 256
N_CORES = 8
BPC = B // N_CORES  # 16 samples per core
MT = TA // 128      # 8 token tiles per sample

_nc_cache = {}
LAST_RESULTS = None  # BassKernelResults of the most recent run (for profiling)


PAIR = 2  # samples per DMA batch (bigger transfers, fewer descriptors)
# float32r: single-pass fp32 matmul mode (4x PE throughput vs the 2-pass
# fp32 path). Bit-identical storage; only the PE multiply path differs.
MM_F32R = True


def _build_nc(repeat=1, pair=None):
    """Build the per-core Bass module. `repeat` re-runs the whole workload
    that many times inside one NEFF (used only for benchmarking: device
    exec time = (T_R - T_1) / (R - 1), cancelling dispatch overhead)."""
    pair = PAIR if pair is None else pair
    key = (repeat, pair, MM_F32R)
    if key in _nc_cache:
        return _nc_cache[key]

    f32 = mybir.dt.float32
    mdt = mybir.dt.float32r if MM_F32R else f32
    nc = bacc.Bacc(
        "TRN2", target_bir_lowering=False, debug=False, enable_asserts=False
    )
    stateT = nc.dram_tensor("stateT", [BPC, IN, TA], mdt, kind="ExternalInput").ap()
    readinT = nc.dram_tensor("readinT", [BPC, RDIM, IN], mdt, kind="ExternalInput").ap()
    proj = nc.dram_tensor("proj", [RDIM, OUT], mdt, kind="ExternalInput").ap()
    out = nc.dram_tensor("out", [BPC, TA, OUT], f32, kind="ExternalOutput").ap()

    with tile.TileContext(nc) as tc, \
         tc.tile_pool(name="const", bufs=1) as cpool, \
         tc.tile_pool(name="w", bufs=3) as wpool, \
         tc.tile_pool(name="s", bufs=3) as spool, \
         tc.tile_pool(name="o", bufs=3) as opool, \
         tc.tile_pool(name="psw", bufs=2, space="PSUM") as pswpool, \
         tc.tile_pool(name="pso", bufs=4, space="PSUM") as psopool:

        proj_sb = cpool.tile([RDIM, OUT], mdt)
        nc.sync.dma_start(proj_sb[:], proj)
        # all 16 samples' transposed readin matrices: [r, b, i], one 768KB DMA
        rT_sb = cpool.tile([RDIM, BPC, IN], mdt)
        nc.sync.dma_start(rT_sb[:], readinT.rearrange("b r i -> r b i"))

        for b0 in [p for _ in range(repeat) for p in range(0, BPC, pair)]:
            # ---- load state.T for `pair` samples ([IN, pair, TA]) ----
            s0 = spool.tile([128, pair, TA], mdt, tag="s0")
            s1 = spool.tile([IN - 128, pair, TA], mdt, tag="s1")
            nc.sync.dma_start(
                s0[:], stateT[b0:b0 + pair, 0:128, :].rearrange("b i t -> i b t"))
            nc.sync.dma_start(
                s1[:], stateT[b0:b0 + pair, 128:IN, :].rearrange("b i t -> i b t"))

            o_sb = opool.tile([128, pair, MT, OUT], f32, tag="o")
            for j in range(pair):
                b = b0 + j
                # ---- fuse W_b = readin_b @ project  ([IN, OUT], K=RDIM) ----
                ps_w0 = pswpool.tile([128, OUT], f32, tag="psw0")
                ps_w1 = pswpool.tile([IN - 128, OUT], f32, tag="psw1")
                nc.tensor.matmul(ps_w0[:], rT_sb[:, b, 0:128], proj_sb[:],
                                 start=True, stop=True)
                nc.tensor.matmul(ps_w1[:], rT_sb[:, b, 128:IN], proj_sb[:],
                                 start=True, stop=True)
                w0 = wpool.tile([128, OUT], mdt, tag="w0")
                w1 = wpool.tile([IN - 128, OUT], mdt, tag="w1")
                nc.scalar.copy(out=w0[:], in_=ps_w0[:])
                nc.scalar.copy(out=w1[:], in_=ps_w1[:])

                # ---- out_b = state_b @ W_b : 8 token tiles, K = 128 + 64 ----
                for mt in range(MT):
                    ps_o = psopool.tile([128, OUT], f32, tag="pso")
                    nc.tensor.matmul(ps_o[:], s0[:, j, ts(mt, 128)], w0[:],
                                     start=True, stop=False)
                    nc.tensor.matmul(ps_o[:], s1[:, j, ts(mt, 128)], w1[:],
                                     start=False, stop=True)
                    nc.vector.tensor_copy(out=o_sb[:, j, mt, :], in_=ps_o[:])
            # store on the ACT HWDGE ring so it doesn't queue behind loads
            nc.scalar.dma_start(
                out[b0:b0 + pair].rearrange("b (mt p) o -> p b mt o", p=128),
                o_sb[:])

    nc.compile()
    _nc_cache[key] = nc
    return nc


def _make_in_maps(state_in, session, unique_readin, project):
    state2d = np.ascontiguousarray(np.asarray(state_in), dtype=np.float32)
    state2d = state2d.reshape(B, TA, IN)
    session_np = np.asarray(session).astype(np.int64)
    table = np.ascontiguousarray(np.asarray(unique_readin), dtype=np.float32)
    proj_np = np.ascontiguousarray(np.asarray(project), dtype=np.float32)

    in_maps = []
    for c in range(N_CORES):
        sl = slice(c * BPC, (c + 1) * BPC)
        stT = np.ascontiguousarray(state2d[sl].transpose(0, 2, 1))
        rT = np.ascontiguousarray(table[session_np[sl]].transpose(0, 2, 1))
        in_maps.append({"stateT": stT, "readinT": rT, "proj": proj_np})
    return in_maps


def kernel(state_in, session, unique_readin, project):
    global LAST_RESULTS
    nc = _build_nc()
    in_maps = _make_in_maps(state_in, session, unique_readin, project)
    res = run_bass_kernel_spmd(nc, in_maps, core_ids=list(range(N_CORES)))
    LAST_RESULTS = res
    outs = [res.results[c]["out"].reshape(BPC, T, A, OUT) for c in range(N_CORES)]
    return np.concatenate(outs, axis=0)
